# revision 1
# baseline (speedup 1.0000x reference)
"""Trainium2 Bass kernel for nn_ProcessContinuous (dense_mlp, memory-bound).

Computation (reference):
    out[m, e*5 + j] = x[m, j] * w_j[e] + (b_j[e] + order_table[j, e])
with (w_j, b_j) for j in 0..4 = (bet, stack, stack, call, odds).

Strategy: pure data-parallel over 8 cores (shard rows M). Per core, rows sit
on SBUF partitions (128 rows/tile); for each tile the 5 output slices
out[:, j::5] are produced by one fused DVE op each:
    scalar_tensor_tensor: (W_j * x[:, j]) + B_j
with W_j/B_j free-dim tables replicated across partitions (loaded once), and
x[:, j] as a per-partition scalar. Each finished [128, 2560] tile is stored
with a single contiguous 1.31 MB DMA (10 KB per HBM row).
"""

import numpy as np

import concourse.bacc as bacc
import concourse.mybir as mybir
from concourse import tile
from concourse.bass_utils import run_bass_kernel_spmd

N_CORES = 8
M = 65536
E = 512
F = 5            # number of scalar features / interleave factor
C = F * E        # 2560 output columns
P = 128          # SBUF partitions
M_LOC = M // N_CORES      # 8192 rows per core
F32 = mybir.dt.float32

_NC_CACHE = {}


def _build(m_loc=M_LOC, out_bufs=6):
    """Build (and cache) the per-core Bass program."""
    key = (m_loc, out_bufs)
    if key in _NC_CACHE:
        return _NC_CACHE[key]

    n_tiles = m_loc // P
    nc = bacc.Bacc(
        "TRN2", target_bir_lowering=False, debug=False, num_devices=N_CORES
    )
    x = nc.dram_tensor("x", [m_loc, F], F32, kind="ExternalInput").ap()
    wtab = nc.dram_tensor("wtab", [P, C], F32, kind="ExternalInput").ap()
    btab = nc.dram_tensor("btab", [P, C], F32, kind="ExternalInput").ap()
    out = nc.dram_tensor("out", [m_loc, C], F32, kind="ExternalOutput").ap()

    # Row m = p*n_tiles + n lives on partition p, tile n. Both the x load and
    # the out store are then contiguous per partition in HBM.
    x_v = x.rearrange("(p n) c -> p (n c)", p=P)      # [128, n_tiles*5]
    out_v = out.rearrange("(p n) c -> p n c", p=P)    # [128, n_tiles, 2560]

    with tile.TileContext(nc) as tc:
        with (
            tc.tile_pool(name="const", bufs=1) as cpool,
            tc.tile_pool(name="outp", bufs=out_bufs) as opool,
        ):
            w_t = cpool.tile([P, C], F32, name="w_t")
            nc.sync.dma_start(out=w_t[:], in_=wtab)
            b_t = cpool.tile([P, C], F32, name="b_t")
            nc.sync.dma_start(out=b_t[:], in_=btab)
            x_t = cpool.tile([P, n_tiles * F], F32, name="x_t")
            nc.sync.dma_start(out=x_t[:], in_=x_v)

            for n in range(n_tiles):
                o_t = opool.tile([P, C], F32, name="o_t", tag="o")
                o_j = o_t[:].rearrange("p (e j) -> p e j", j=F)
                for j in range(F):
                    nc.vector.scalar_tensor_tensor(
                        out=o_j[:, :, j],
                        in0=w_t[:, j * E : (j + 1) * E],
                        scalar=x_t[:, n * F + j : n * F + j + 1],
                        in1=b_t[:, j * E : (j + 1) * E],
                        op0=mybir.AluOpType.mult,
                        op1=mybir.AluOpType.add,
                    )
                nc.sync.dma_start(out=out_v[:, n, :], in_=o_t[:])

    nc.compile()
    _NC_CACHE[key] = nc
    return nc


def _tables(w_bet, b_bet, w_stack, b_stack, w_call, b_call, w_odds, b_odds,
            order_table):
    """Planar weight/bias tables [w_0|...|w_4], replicated across partitions."""
    wp = np.stack([w_bet, w_stack, w_stack, w_call, w_odds]).astype(np.float32)
    bp = np.stack([b_bet, b_stack, b_stack, b_call, b_odds]).astype(
        np.float32
    ) + np.asarray(order_table, np.float32)
    wtab = np.ascontiguousarray(np.broadcast_to(wp.reshape(1, C), (P, C)))
    btab = np.ascontiguousarray(np.broadcast_to(bp.reshape(1, C), (P, C)))
    return wtab, btab


def _run(x, wtab, btab, trace=False, **kwargs):
    x = np.ascontiguousarray(np.asarray(x, np.float32))
    nc = _build()
    in_maps = [
        {"x": x[c * M_LOC : (c + 1) * M_LOC], "wtab": wtab, "btab": btab}
        for c in range(N_CORES)
    ]
    return run_bass_kernel_spmd(
        nc, in_maps, list(range(N_CORES)), trace=trace, **kwargs
    )


def kernel(x, w_bet, b_bet, w_stack, b_stack, w_call, b_call, w_odds, b_odds,
           order_table):
    wtab, btab = _tables(
        w_bet, b_bet, w_stack, b_stack, w_call, b_call, w_odds, b_odds,
        order_table,
    )
    res = _run(x, wtab, btab).results
    return np.concatenate([res[c]["out"] for c in range(N_CORES)], axis=0)


# revision 2
# speedup vs baseline: 1.3246x; 1.3246x over previous
"""Trainium2 Bass kernel for nn_ProcessContinuous (dense_mlp, memory-bound).

Computation (reference):
    out[m, e*5 + j] = x[m, j] * w_j[e] + (b_j[e] + order_table[j, e])
with (w_j, b_j) for j in 0..4 = (bet, stack, stack, call, odds).

Strategy: pure data-parallel over 8 cores (shard rows M; 8192 rows/core).
Per core, the whole affine map is ONE tiny-K matmul per 128-row tile,
computed exactly in fp32 via 3-way bf16 splitting:
    x = xh + xm + xl,  W = Wh + Wm + Wl   (exact fp32 decompositions)
    x*W = sum_{a,b} x_a * W_b             (each product exact in fp32 MAC)
K = 5 cols * 9 split-pairs + 3 bias rows = 48. The rhs [48, 2560] bakes in
the output interleave and the fused bias (b_j + order_table), so the PE
writes final interleaved values into PSUM. The 5 [128,512] PSUM banks per
tile are copied to SBUF alternating between DVE and ACT (~670 ns each),
and each finished [128, 2560] tile is stored with a single fully
contiguous 1.31 MB DMA. HBM write bandwidth is the roofline.
"""

import numpy as np
import ml_dtypes

import concourse.bacc as bacc
import concourse.mybir as mybir
from concourse import tile
from concourse.bass_utils import run_bass_kernel_spmd

N_CORES = 8
M = 65536
E = 512
F = 5            # number of scalar features / interleave factor
C = F * E        # 2560 output columns
P = 128          # SBUF partitions
K = 48           # 5 cols * 9 bf16 split-pairs + 3 bias rows
M_LOC = M // N_CORES      # 8192 rows per core
F32 = mybir.dt.float32
BF16 = mybir.dt.bfloat16

_NC_CACHE = {}


def _build(m_loc=M_LOC, out_bufs=6):
    """Build (and cache) the per-core Bass program."""
    key = (m_loc, out_bufs)
    if key in _NC_CACHE:
        return _NC_CACHE[key]

    n_tiles = m_loc // P
    nc = bacc.Bacc(
        "TRN2", target_bir_lowering=False, debug=False, num_devices=N_CORES
    )
    xs = nc.dram_tensor("xs", [K, m_loc], BF16, kind="ExternalInput").ap()
    rhs = nc.dram_tensor("rhs", [K, C], BF16, kind="ExternalInput").ap()
    out = nc.dram_tensor("out", [m_loc, C], F32, kind="ExternalOutput").ap()

    # Row m = n*128 + p: tile n is a contiguous 128-row block. The output
    # DMA per tile is one fully contiguous 1.31 MB block in HBM.
    out_v = out.rearrange("(n p) c -> p n c", p=P)    # [128, n_tiles, 2560]

    with tile.TileContext(nc) as tc:
        with (
            tc.tile_pool(name="const", bufs=1) as cpool,
            tc.tile_pool(name="outp", bufs=out_bufs) as opool,
            tc.tile_pool(name="ps", bufs=8, space="PSUM") as ppool,
        ):
            xs_t = cpool.tile([K, m_loc], BF16, name="xs_t")
            nc.sync.dma_start(out=xs_t[:], in_=xs)
            rhs_t = cpool.tile([K, C], BF16, name="rhs_t")
            nc.sync.dma_start(out=rhs_t[:], in_=rhs)

            for n in range(n_tiles):
                o_t = opool.tile([P, C], F32, name="o_t", tag="o")
                lhsT = xs_t[:, n * P : (n + 1) * P]
                for c in range(F):
                    ps = ppool.tile([P, E], F32, name="ps", tag="ps")
                    nc.tensor.matmul(
                        ps[:], lhsT, rhs_t[:, c * E : (c + 1) * E],
                        start=True, stop=True,
                    )
                    dst = o_t[:, c * E : (c + 1) * E]
                    if (n * F + c) % 2 == 0:
                        nc.vector.tensor_copy(out=dst, in_=ps[:])
                    else:
                        nc.scalar.activation(
                            dst, ps[:], mybir.ActivationFunctionType.Copy
                        )
                nc.sync.dma_start(out=out_v[:, n, :], in_=o_t[:])

    nc.compile()
    _NC_CACHE[key] = nc
    return nc


def _split3(a):
    """Exact 3-way bf16 decomposition of fp32 array a: a == h + m + l."""
    a = np.asarray(a, np.float32)
    h = a.astype(ml_dtypes.bfloat16)
    r = a - h.astype(np.float32)
    m = r.astype(ml_dtypes.bfloat16)
    l = (r - m.astype(np.float32)).astype(ml_dtypes.bfloat16)
    return h, m, l


def _tables(w_bet, b_bet, w_stack, b_stack, w_call, b_call, w_odds, b_odds,
            order_table):
    """rhs [48, 2560] bf16: interleaved W splits per source col + bias rows."""
    wp = np.stack([w_bet, w_stack, w_stack, w_call, w_odds]).astype(np.float32)
    bp = np.stack([b_bet, b_stack, b_stack, b_call, b_odds]).astype(
        np.float32
    ) + np.asarray(order_table, np.float32)
    w_int = np.ascontiguousarray(wp.T).reshape(C)   # w_int[e*5+j] = w_j[e]
    b_int = np.ascontiguousarray(bp.T).reshape(C)
    w_sp = _split3(w_int)                            # 3 x [2560] bf16
    b_sp = _split3(b_int)
    rhs = np.zeros((K, C), dtype=ml_dtypes.bfloat16)
    k_idx = np.arange(C)
    for c in range(F):
        mask = (k_idx % F) == c
        for a in range(3):
            for b in range(3):
                rhs[c * 9 + a * 3 + b, mask] = w_sp[b][mask]
    for s in range(3):
        rhs[45 + s] = b_sp[s]
    return rhs


def _lhs(x):
    """xs [48, m] bf16: x-split rows matching _tables' pair layout."""
    x = np.asarray(x, np.float32)
    m = x.shape[0]
    x_sp = _split3(x)                                # 3 x [m, 5] bf16
    xs = np.zeros((K, m), dtype=ml_dtypes.bfloat16)
    for c in range(F):
        for a in range(3):
            for b in range(3):
                xs[c * 9 + a * 3 + b] = x_sp[a][:, c]
    xs[45:48] = 1.0
    return xs


def _run(x, rhs, trace=False, **kwargs):
    x = np.ascontiguousarray(np.asarray(x, np.float32))
    nc = _build()
    in_maps = []
    for c in range(N_CORES):
        xs = _lhs(x[c * M_LOC : (c + 1) * M_LOC])
        in_maps.append({"xs": xs, "rhs": rhs})
    return run_bass_kernel_spmd(
        nc, in_maps, list(range(N_CORES)), trace=trace, **kwargs
    )


def kernel(x, w_bet, b_bet, w_stack, b_stack, w_call, b_call, w_odds, b_odds,
           order_table):
    rhs = _tables(
        w_bet, b_bet, w_stack, b_stack, w_call, b_call, w_odds, b_odds,
        order_table,
    )
    res = _run(x, rhs).results
    return np.concatenate([res[c]["out"] for c in range(N_CORES)], axis=0)


# revision 5
# speedup vs baseline: 1.4602x; 1.1023x over previous
"""Trainium2 Bass kernel for nn_ProcessContinuous (dense_mlp, memory-bound).

Computation (reference):
    out[m, e*5 + j] = x[m, j] * w_j[e] + (b_j[e] + order_table[j, e])
with (w_j, b_j) for j in 0..4 = (bet, stack, stack, call, odds).

Strategy: pure data-parallel over 8 cores (shard rows M; 8192 rows/core).
Per core, the whole affine map is ONE tiny-K matmul per 128-row tile,
computed exactly in fp32 via 3-way bf16 splitting:
    x = xh + xm + xl,  W = Wh + Wm + Wl   (exact fp32 decompositions)
    x*W = sum_{a,b} x_a * W_b             (each product exact in fp32 MAC)
K = 5 cols * 9 split-pairs + 3 bias rows = 48. The rhs [48, 2560] bakes in
the output interleave and the fused bias (b_j + order_table), so the PE
writes final interleaved values into PSUM. The 5 [128,512] PSUM banks per
tile are copied to SBUF alternating between DVE and ACT (~670 ns each),
and each finished [128, 2560] tile is stored with a single fully
contiguous 1.31 MB DMA. HBM write bandwidth is the roofline.
"""

import numpy as np
import ml_dtypes

import concourse.bacc as bacc
import concourse.mybir as mybir
from concourse import tile
from concourse.bass_utils import run_bass_kernel_spmd

N_CORES = 8
M = 65536
E = 512
F = 5            # number of scalar features / interleave factor
C = F * E        # 2560 output columns
P = 128          # SBUF partitions
K = 48           # 5 cols * 9 bf16 split-pairs + 3 bias rows
M_LOC = M // N_CORES      # 8192 rows per core
F32 = mybir.dt.float32
BF16 = mybir.dt.bfloat16

_NC_CACHE = {}


def _build(m_loc=M_LOC, out_bufs=8, group=1):
    """Build (and cache) the per-core Bass program.

    group: number of 128-row tiles per output SBUF slot / store DMA.
    """
    key = (m_loc, out_bufs, group)
    if key in _NC_CACHE:
        return _NC_CACHE[key]

    n_tiles = m_loc // P
    n_groups = n_tiles // group
    nc = bacc.Bacc(
        "TRN2", target_bir_lowering=False, debug=False, num_devices=N_CORES
    )
    xs = nc.dram_tensor("xs", [K, m_loc], BF16, kind="ExternalInput").ap()
    rhs = nc.dram_tensor("rhs", [K, C], BF16, kind="ExternalInput").ap()
    out = nc.dram_tensor("out", [m_loc, C], F32, kind="ExternalOutput").ap()

    # Row m = n*128 + p: tile n is a contiguous 128-row block; a group of
    # `group` consecutive tiles is one contiguous HBM region per partition.
    out_v = out.rearrange("(g t p) c -> p g t c", p=P, t=group)

    with tile.TileContext(nc) as tc:
        with (
            tc.tile_pool(name="const", bufs=1) as cpool,
            tc.tile_pool(name="outp", bufs=out_bufs) as opool,
            tc.tile_pool(name="ps", bufs=8, space="PSUM") as ppool,
        ):
            xs_t = cpool.tile([K, m_loc], BF16, name="xs_t")
            nc.sync.dma_start(out=xs_t[:], in_=xs)
            rhs_t = cpool.tile([K, C], BF16, name="rhs_t")
            nc.sync.dma_start(out=rhs_t[:], in_=rhs)

            for g in range(n_groups):
                o_t = opool.tile([P, group * C], F32, name="o_t", tag="o")
                for t in range(group):
                    n = g * group + t
                    lhsT = xs_t[:, n * P : (n + 1) * P]
                    for c in range(F):
                        ps = ppool.tile([P, E], F32, name="ps", tag="ps")
                        nc.tensor.matmul(
                            ps[:], lhsT, rhs_t[:, c * E : (c + 1) * E],
                            start=True, stop=True,
                        )
                        dst = o_t[:, t * C + c * E : t * C + (c + 1) * E]
                        if (n * F + c) % 2 == 0:
                            nc.vector.tensor_copy(out=dst, in_=ps[:])
                        else:
                            nc.scalar.activation(
                                dst, ps[:], mybir.ActivationFunctionType.Copy
                            )
                nc.sync.dma_start(out=out_v[:, g], in_=o_t[:])

    nc.compile()
    _NC_CACHE[key] = nc
    return nc


def _split3(a):
    """Exact 3-way bf16 decomposition of fp32 array a: a == h + m + l."""
    a = np.asarray(a, np.float32)
    h = a.astype(ml_dtypes.bfloat16)
    r = a - h.astype(np.float32)
    m = r.astype(ml_dtypes.bfloat16)
    l = (r - m.astype(np.float32)).astype(ml_dtypes.bfloat16)
    return h, m, l


def _tables(w_bet, b_bet, w_stack, b_stack, w_call, b_call, w_odds, b_odds,
            order_table):
    """rhs [48, 2560] bf16: interleaved W splits per source col + bias rows."""
    wp = np.stack([w_bet, w_stack, w_stack, w_call, w_odds]).astype(np.float32)
    bp = np.stack([b_bet, b_stack, b_stack, b_call, b_odds]).astype(
        np.float32
    ) + np.asarray(order_table, np.float32)
    w_int = np.ascontiguousarray(wp.T).reshape(C)   # w_int[e*5+j] = w_j[e]
    b_int = np.ascontiguousarray(bp.T).reshape(C)
    w_sp = _split3(w_int)                            # 3 x [2560] bf16
    b_sp = _split3(b_int)
    rhs = np.zeros((K, C), dtype=ml_dtypes.bfloat16)
    k_idx = np.arange(C)
    for c in range(F):
        mask = (k_idx % F) == c
        for a in range(3):
            for b in range(3):
                rhs[c * 9 + a * 3 + b, mask] = w_sp[b][mask]
    for s in range(3):
        rhs[45 + s] = b_sp[s]
    return rhs


def _lhs(x):
    """xs [48, m] bf16: x-split rows matching _tables' pair layout."""
    x = np.asarray(x, np.float32)
    m = x.shape[0]
    x_sp = _split3(x)                                # 3 x [m, 5] bf16
    xs = np.zeros((K, m), dtype=ml_dtypes.bfloat16)
    for c in range(F):
        for a in range(3):
            for b in range(3):
                xs[c * 9 + a * 3 + b] = x_sp[a][:, c]
    xs[45:48] = 1.0
    return xs


def _run(x, rhs, trace=False, build_kwargs=None, **kwargs):
    x = np.ascontiguousarray(np.asarray(x, np.float32))
    nc = _build(**(build_kwargs or {}))
    in_maps = []
    for c in range(N_CORES):
        xs = _lhs(x[c * M_LOC : (c + 1) * M_LOC])
        in_maps.append({"xs": xs, "rhs": rhs})
    return run_bass_kernel_spmd(
        nc, in_maps, list(range(N_CORES)), trace=trace, **kwargs
    )


def kernel(x, w_bet, b_bet, w_stack, b_stack, w_call, b_call, w_odds, b_odds,
           order_table):
    rhs = _tables(
        w_bet, b_bet, w_stack, b_stack, w_call, b_call, w_odds, b_odds,
        order_table,
    )
    res = _run(x, rhs).results
    return np.concatenate([res[c]["out"] for c in range(N_CORES)], axis=0)
